# revision 3
# baseline (speedup 1.0000x reference)
"""
LoRA-Quant-Linear Trainium2 kernel (8 NeuronCores).

Math:  out = x @ W^T + bias + LORA_SCALE * ((x @ a^T) @ b^T)
       a = qa * scale_a  [16, 4096],  b = qb * scale_b  [4096, 16]

Sharding (2 batch-groups x 4 out-column-groups = 8 cores):
  core c = (mg, ng), mg = c // 4, ng = c % 4
    - x rows   [mg*8192 : (mg+1)*8192]  (of B*S = 16384), host-transposed -> xT [4096, 8192]
    - W rows   [ng*1024 : (ng+1)*1024]  (out_features),   host-transposed -> wT [4096, 1024]
  LoRA is folded into the weight chunk on the host (out = x @ (W^T + s*a^T b^T) + bias,
  exact associativity; the fold is 0.4% of the FLOPs).

The GEMM is PE-bound (68.7 GFLOP/core; 1 cycle/row at 2.4 GHz => ~874 us floor),
so the kernel is organized to keep the PE saturated end to end:
  - x and W_eff are cast to bf16 on the host (error ~2e-3 of out scale, budget
    is 2e-2).  bf16 halves the HBM stream and enables fast weight load.
  - a short warm-up matmul chain on zeros runs while the first DMAs fill,
    covering the HAM clock-gate ramp (~3.4 us at half clock) and pstate ramp.
  - the first 4 m-slivers are processed kt-major across all 8 PSUM banks, so
    the PE only ever needs W k-slices that have already arrived while the
    16->8 MiB W residency load streams in; remaining 60 slivers run mt-major
    with double-buffered PSUM pairs.
  - x stream on the scalar DMA queue, W on sync, out on gpsimd: no queue
    head-of-line blocking between the three flows.
"""

import numpy as np

LORA_SCALE = 32.0 / 16.0

P = 128
K = 4096            # contraction dim (D_in)
KT = K // P         # 32 k-tiles
M_CORE = 8192       # x rows per core
N_CORE = 1024       # out columns per core
MT = M_CORE // P    # 64 m-slivers
NB = 512            # moving free dim per matmul (PSUM bank = 512 f32)
NH = N_CORE // NB   # 2
N_CORES = 8
MG, NG = 2, 4       # core grid
MT_P1 = 4           # slivers processed kt-major in phase 1 (8 PSUM banks)
WARMUP_MM = 16      # warm-up matmuls (~3.4 us) hidden under the initial fill
P1_OFF = 7          # phase-1 diagonal stagger: chain m joins after m*P1_OFF steps

_CACHE = {}


def _build_program():
    import concourse.tile as tile
    from concourse import bacc, mybir
    from contextlib import ExitStack

    f32 = mybir.dt.float32
    bf16 = mybir.dt.bfloat16

    nc = bacc.Bacc("TRN2", target_bir_lowering=False, debug=False,
                   num_devices=N_CORES)

    # host-pretiled layouts: xT[mt, p, kt, ml] = x[mt*128+ml, kt*128+p]
    # (per-sliver contiguous => 8 KiB/partition DMA lines), and
    # wT[p, kt, n] = W_eff^T[kt*128+p, n].
    xT = nc.dram_tensor("xT", [MT, P, KT, P], bf16, kind="ExternalInput").ap()
    wT = nc.dram_tensor("wT", [P, KT, N_CORE], bf16, kind="ExternalInput").ap()
    biasb = nc.dram_tensor("biasb", [P, N_CORE], f32, kind="ExternalInput").ap()
    out = nc.dram_tensor("out", [M_CORE, N_CORE], f32, kind="ExternalOutput").ap()

    out_t = out.rearrange("(mt p) n -> mt p n", p=P)    # [64, 128, 1024]

    with tile.TileContext(nc) as tc, ExitStack() as ctx:
        wpool = ctx.enter_context(tc.tile_pool(name="wres", bufs=1))
        cpool = ctx.enter_context(tc.tile_pool(name="consts", bufs=1))
        xpool = ctx.enter_context(tc.tile_pool(name="xs", bufs=8))
        opool = ctx.enter_context(tc.tile_pool(name="outs", bufs=4))
        pspool = ctx.enter_context(tc.tile_pool(name="ps", bufs=8, space="PSUM"))

        # ---- warm-up: keep the PE busy (and the clock gate open) while the
        # first x slivers + W slices stream in.  Zeros in, zeros out; the
        # single DVE copy consumer lets the PSUM bank recycle.
        wz_st = cpool.tile([P, P], bf16)
        wz_mv = cpool.tile([P, NB], bf16)
        wz_out = cpool.tile([P, NB], f32)
        nc.vector.memset(wz_st[:], 0.0)
        nc.vector.memset(wz_mv[:], 0.0)
        warm_ps = pspool.tile([P, NB], f32, tag="ps", name="ps_warm")
        for i in range(WARMUP_MM):
            nc.tensor.matmul(warm_ps[:], wz_st[:], wz_mv[:],
                             start=(i == 0), stop=(i == WARMUP_MM - 1))
        nc.vector.tensor_copy(wz_out[:], warm_ps[:])

        # ---- fill: first MT_P1 x slivers, bias, then resident fused weights
        x_sbs = {}
        for mt in range(MT_P1):
            x_sbs[mt] = xpool.tile([P, KT, P], bf16, tag="x", name=f"x_{mt}")
            nc.scalar.dma_start(x_sbs[mt][:], xT[mt])

        bias_sb = cpool.tile([P, N_CORE], f32)
        nc.scalar.dma_start(bias_sb[:], biasb)

        w_sb = wpool.tile([P, KT, N_CORE], bf16)
        for kt in range(KT):
            nc.sync.dma_start(w_sb[:, kt, :], wT[:, kt, :])

        def drain(mt, pss):
            o_sb = opool.tile([P, N_CORE], f32, tag="o", name=f"o_{mt}")
            for nh in range(NH):
                nc.vector.tensor_add(
                    o_sb[:, nh * NB:(nh + 1) * NB],
                    pss[nh][:],
                    bias_sb[:, nh * NB:(nh + 1) * NB],
                )
            nc.gpsimd.dma_start(out_t[mt], o_sb[:])

        # ---- phase 1: kt-major over the first MT_P1 slivers.  Each W k-slice
        # is used for 8 matmuls (~1.7 us) as soon as it lands, so the PE never
        # outruns the W residency load.
        ps1 = {(m, h): pspool.tile([P, NB], f32, tag="ps", name=f"ps1_{m}_{h}")
               for m in range(MT_P1) for h in range(NH)}
        for kt in range(KT):
            for m in range(MT_P1):
                for h in range(NH):
                    nc.tensor.matmul(
                        ps1[(m, h)][:],
                        x_sbs[m][:, kt, :],
                        w_sb[:, kt, h * NB:(h + 1) * NB],
                        start=(kt == 0), stop=(kt == KT - 1),
                    )
        for m in range(MT_P1):
            drain(m, [ps1[(m, 0)], ps1[(m, 1)]])

        # ---- phase 2: mt-major, streaming x slivers through resident W.
        for mt in range(MT_P1, MT):
            x_sb = xpool.tile([P, KT, P], bf16, tag="x", name=f"x_{mt}")
            nc.scalar.dma_start(x_sb[:], xT[mt])
            pss = [pspool.tile([P, NB], f32, tag="ps", name=f"ps_{mt}_{i}")
                   for i in range(NH)]
            for kt in range(KT):
                for nh in range(NH):
                    nc.tensor.matmul(
                        pss[nh][:],
                        x_sb[:, kt, :],
                        w_sb[:, kt, nh * NB:(nh + 1) * NB],
                        start=(kt == 0), stop=(kt == KT - 1),
                    )
            drain(mt, pss)

    nc.compile()
    return nc


def _get_program():
    if "nc" not in _CACHE:
        _CACHE["nc"] = _build_program()
    return _CACHE["nc"]


def _make_in_maps(x, W, bias, qa, qb, scale_a, scale_b):
    import ml_dtypes

    bf16 = ml_dtypes.bfloat16
    x2 = np.ascontiguousarray(x.reshape(MG * M_CORE, K))
    a_deq = qa.astype(np.float32) * np.float32(scale_a)       # [16, 4096]
    b_deq = qb.astype(np.float32) * np.float32(scale_b)       # [4096, 16]
    # W_eff^T = W^T + s * a^T @ b^T   -> [K, N_full]
    w_eff_T = W.T + np.float32(LORA_SCALE) * (a_deq.T @ b_deq.T)
    bias = bias.astype(np.float32)

    # [mt, ml, kt, p] -> [mt, p, kt, ml], cast to bf16
    xT_by_mg = [np.ascontiguousarray(
                    x2[mg * M_CORE:(mg + 1) * M_CORE, :]
                    .reshape(MT, P, KT, P).transpose(0, 3, 2, 1)
                    .astype(bf16))
                for mg in range(MG)]
    in_maps = []
    for c in range(N_CORES):
        mg, ng = c // NG, c % NG
        nsl = slice(ng * N_CORE, (ng + 1) * N_CORE)
        in_maps.append({
            "xT": xT_by_mg[mg],
            "wT": np.ascontiguousarray(
                w_eff_T[:, nsl].reshape(KT, P, N_CORE).transpose(1, 0, 2)
                .astype(bf16)),
            "biasb": np.ascontiguousarray(
                np.broadcast_to(bias[nsl], (P, N_CORE))),
        })
    return in_maps


def kernel(x, W, bias, qa, qb, scale_a, scale_b, _trace=False):
    from concourse.bass_utils import run_bass_kernel_spmd

    nc = _get_program()
    in_maps = _make_in_maps(np.asarray(x, dtype=np.float32),
                            np.asarray(W, dtype=np.float32),
                            np.asarray(bias, dtype=np.float32),
                            np.asarray(qa), np.asarray(qb),
                            np.asarray(scale_a), np.asarray(scale_b))
    res = run_bass_kernel_spmd(nc, in_maps, core_ids=list(range(N_CORES)),
                               trace=_trace)
    B, S = 4, 4096
    full = np.empty((MG * M_CORE, NG * N_CORE), dtype=np.float32)
    for c in range(N_CORES):
        mg, ng = c // NG, c % NG
        full[mg * M_CORE:(mg + 1) * M_CORE,
             ng * N_CORE:(ng + 1) * N_CORE] = res.results[c]["out"]
    if _trace:
        kernel._last_results = res
    return full.reshape(B, S, K)


# revision 10
# speedup vs baseline: 1.4279x; 1.4279x over previous
"""
LoRA-Quant-Linear Trainium2 kernel (8 NeuronCores).

Math:  out = x @ W^T + bias + LORA_SCALE * ((x @ a^T) @ b^T)
       a = qa * scale_a  [16, 4096],  b = qb * scale_b  [4096, 16]

Sharding (2 batch-groups x 4 out-column-groups = 8 cores):
  core c = (mg, ng), mg = c // 4, ng = c % 4
    - x rows   [mg*8192 : (mg+1)*8192]  (of B*S = 16384), host-transposed -> xT [4096, 8192]
    - W rows   [ng*1024 : (ng+1)*1024]  (out_features),   host-transposed -> wT [4096, 1024]
  LoRA is folded into the weight chunk on the host (out = x @ (W^T + s*a^T b^T) + bias,
  exact associativity; the fold is 0.4% of the FLOPs).

The GEMM is PE-bound (68.7 GFLOP/core; 1 cycle/row at 2.4 GHz => ~874 us floor),
so the kernel is organized to keep the PE saturated end to end:
  - x and W_eff are cast to bf16 on the host (error ~1.3e-3 of out scale, budget
    is 2e-2).  bf16 halves the HBM stream and enables fast weight load.
  - a short warm-up matmul chain on zeros runs while the first DMAs fill,
    covering the HAM clock-gate ramp (~3.4 us at half clock) and pstate ramp.
  - the first 4 m-slivers are processed kt-major across all 8 PSUM banks with
    a diagonal stagger (chain m joins P1_OFF k-steps after chain m-1), so the
    PE only ever needs data that has already arrived while the 8 MiB W
    residency load and the first x slivers stream in; the remaining 60
    slivers run mt-major with the PSUM pool double-buffering pairs.
  - the last sliver runs nh-outer so its first half drains (bias add + store)
    under the second half's matmuls, shrinking the end-of-kernel tail.
  - x stream on the scalar DMA queue (HWDGE); W residency and out stores
    share the sync queue's hardware DGE without overlapping in time (W done
    ~40 us in, stores start ~60 us in).  gpsimd/SWDGE is avoided: its
    software descriptor build costs ~3.4 us per store and sits on the
    end-of-kernel critical path.
"""

import numpy as np

LORA_SCALE = 32.0 / 16.0

P = 128
K = 4096            # contraction dim (D_in)
KT = K // P         # 32 k-tiles
M_CORE = 8192       # x rows per core
N_CORE = 1024       # out columns per core
MT = M_CORE // P    # 64 m-slivers
NB = 512            # moving free dim per matmul (PSUM bank = 512 f32)
NH = N_CORE // NB   # 2
N_CORES = 8
MG, NG = 2, 4       # core grid
MT_P1 = 4           # slivers processed kt-major in phase 1 (8 PSUM banks)
WARMUP_MM = 24      # warm-up matmuls (~5 us) hidden under the initial fill
P1_OFF = 0          # phase-1 stagger disabled: the 2-HWDGE-queue bus split,
                    # not emission order, binds the early schedule (measured)

_CACHE = {}


def _emit_body(nc, tc, pools, aps, it=""):
    """One full pass of the kernel.  ``it`` uniquifies tile names when the
    body is emitted more than once (timing harness)."""
    from concourse import mybir

    f32 = mybir.dt.float32
    bf16 = mybir.dt.bfloat16
    wpool, cpool, xpool, opool, pspool = pools
    xT, wT, biasb, out_t = aps

    # ---- warm-up: keep the PE busy (and the clock gate open) while the
    # first x slivers + W slices stream in.
    wz = cpool.tile([P, NB], bf16, tag="wz", name=f"wz{it}")
    wz_out = cpool.tile([P, NB], f32, tag="wz_out", name=f"wz_out{it}")
    nc.vector.memset(wz[:], 0.0)
    warm_ps = pspool.tile([P, NB], f32, tag="ps", name=f"ps_warm{it}")
    for i in range(WARMUP_MM):
        nc.tensor.matmul(warm_ps[:], wz[:, :P], wz[:],
                         start=(i == 0), stop=(i == WARMUP_MM - 1))
    nc.vector.tensor_copy(wz_out[:], warm_ps[:])

    # ---- fill: first MT_P1 x slivers on the scalar queue, W residency on
    # sync, bias on the (otherwise idle) gpsimd queue.
    x_sbs = {}
    for mt in range(MT_P1):
        x_sbs[mt] = xpool.tile([P, KT, P], bf16, tag="x", name=f"x_{mt}{it}")
        nc.scalar.dma_start(x_sbs[mt][:], xT[mt])

    # bias is first needed at the phase-1 drain (~60 us in); keep it on the
    # scalar queue behind the phase-1 slivers so it doesn't steal early bus.
    bias_sb = cpool.tile([P, N_CORE], f32, tag="bias", name=f"bias{it}")
    nc.scalar.dma_start(bias_sb[:], biasb)

    w_sb = wpool.tile([P, KT, N_CORE], bf16, tag="w", name=f"w{it}")
    for kt in range(KT):
        nc.sync.dma_start(w_sb[:, kt, :], wT[:, kt, :])

    def drain(mt, pss):
        o_sb = opool.tile([P, N_CORE], f32, tag="o", name=f"o_{mt}{it}")
        for nh in range(NH):
            nc.vector.tensor_add(
                o_sb[:, nh * NB:(nh + 1) * NB],
                pss[nh][:],
                bias_sb[:, nh * NB:(nh + 1) * NB],
            )
        nc.sync.dma_start(out_t[mt], o_sb[:])

    # ---- phase 1: kt-major over the first MT_P1 slivers, diagonally
    # staggered.  Each W k-slice is consumed over ~1.7 us once all chains are
    # active, so the PE never outruns the W residency load, and chain m only
    # needs x sliver m after ~m*P1_OFF*0.85 us.
    ps1 = {(m, h): pspool.tile([P, NB], f32, tag="ps", name=f"ps1_{m}_{h}{it}")
           for m in range(MT_P1) for h in range(NH)}
    for s in range(KT + (MT_P1 - 1) * P1_OFF):
        for m in range(MT_P1):
            kt = s - m * P1_OFF
            if 0 <= kt < KT:
                for h in range(NH):
                    nc.tensor.matmul(
                        ps1[(m, h)][:],
                        x_sbs[m][:, kt, :],
                        w_sb[:, kt, h * NB:(h + 1) * NB],
                        start=(kt == 0), stop=(kt == KT - 1),
                    )
    for m in range(MT_P1):
        drain(m, [ps1[(m, 0)], ps1[(m, 1)]])

    # ---- phase 2: mt-major, streaming x slivers through resident W.
    for mt in range(MT_P1, MT):
        x_sb = xpool.tile([P, KT, P], bf16, tag="x", name=f"x_{mt}{it}")
        nc.scalar.dma_start(x_sb[:], xT[mt])
        pss = [pspool.tile([P, NB], f32, tag="ps", name=f"ps_{mt}_{i}{it}")
               for i in range(NH)]
        if mt < MT - 1:
            for kt in range(KT):
                for nh in range(NH):
                    nc.tensor.matmul(
                        pss[nh][:],
                        x_sb[:, kt, :],
                        w_sb[:, kt, nh * NB:(nh + 1) * NB],
                        start=(kt == 0), stop=(kt == KT - 1),
                    )
            drain(mt, pss)
        else:
            # last sliver: four quarter-width chains, each draining (bias add
            # + store) under the next chain's matmuls, so the end-of-kernel
            # tail is just one quarter drain.
            NQ = NB // 2
            o_sb = opool.tile([P, N_CORE], f32, tag="o", name=f"o_{mt}{it}")
            qss = [pspool.tile([P, NQ], f32, tag="ps", name=f"psq_{q}{it}")
                   for q in range(4)]
            for q in range(4):
                for kt in range(KT):
                    nc.tensor.matmul(
                        qss[q][:],
                        x_sb[:, kt, :],
                        w_sb[:, kt, q * NQ:(q + 1) * NQ],
                        start=(kt == 0), stop=(kt == KT - 1),
                    )
                nc.vector.tensor_add(
                    o_sb[:, q * NQ:(q + 1) * NQ],
                    qss[q][:],
                    bias_sb[:, q * NQ:(q + 1) * NQ],
                )
                nc.sync.dma_start(out_t[mt][:, q * NQ:(q + 1) * NQ],
                                  o_sb[:, q * NQ:(q + 1) * NQ])


def _build_program(reps=1):
    import concourse.tile as tile
    from concourse import bacc, mybir
    from contextlib import ExitStack

    f32 = mybir.dt.float32
    bf16 = mybir.dt.bfloat16

    nc = bacc.Bacc("TRN2", target_bir_lowering=False, debug=False,
                   num_devices=N_CORES)

    # host-pretiled layouts: xT[mt, p, kt, ml] = x[mt*128+ml, kt*128+p]
    # (per-sliver contiguous => 8 KiB/partition DMA lines), and
    # wT[p, kt, n] = W_eff^T[kt*128+p, n].
    xT = nc.dram_tensor("xT", [MT, P, KT, P], bf16, kind="ExternalInput").ap()
    wT = nc.dram_tensor("wT", [P, KT, N_CORE], bf16, kind="ExternalInput").ap()
    biasb = nc.dram_tensor("biasb", [P, N_CORE], f32, kind="ExternalInput").ap()
    out = nc.dram_tensor("out", [M_CORE, N_CORE], f32, kind="ExternalOutput").ap()

    out_t = out.rearrange("(mt p) n -> mt p n", p=P)    # [64, 128, 1024]

    with tile.TileContext(nc) as tc, ExitStack() as ctx:
        pools = (
            ctx.enter_context(tc.tile_pool(name="wres", bufs=1)),
            ctx.enter_context(tc.tile_pool(name="consts", bufs=1)),
            ctx.enter_context(tc.tile_pool(name="xs", bufs=8)),
            ctx.enter_context(tc.tile_pool(name="outs", bufs=4)),
            ctx.enter_context(tc.tile_pool(name="ps", bufs=8, space="PSUM")),
        )
        aps = (xT, wT, biasb, out_t)
        if reps == 1:
            _emit_body(nc, tc, pools, aps)
        else:
            from concourse.mybir import EngineType
            with tc.For_i(0, reps, 1, hint_engines=(EngineType.PE,)):
                _emit_body(nc, tc, pools, aps, it="_r")

    nc.compile()
    return nc


def _get_program():
    if "nc" not in _CACHE:
        _CACHE["nc"] = _build_program()
    return _CACHE["nc"]


def _make_in_maps(x, W, bias, qa, qb, scale_a, scale_b):
    import ml_dtypes

    bf16 = ml_dtypes.bfloat16
    x2 = np.ascontiguousarray(x.reshape(MG * M_CORE, K))
    a_deq = qa.astype(np.float32) * np.float32(scale_a)       # [16, 4096]
    b_deq = qb.astype(np.float32) * np.float32(scale_b)       # [4096, 16]
    # W_eff^T = W^T + s * a^T @ b^T   -> [K, N_full]
    w_eff_T = W.T + np.float32(LORA_SCALE) * (a_deq.T @ b_deq.T)
    bias = bias.astype(np.float32)

    # [mt, ml, kt, p] -> [mt, p, kt, ml], cast to bf16
    xT_by_mg = [np.ascontiguousarray(
                    x2[mg * M_CORE:(mg + 1) * M_CORE, :]
                    .reshape(MT, P, KT, P).transpose(0, 3, 2, 1)
                    .astype(bf16))
                for mg in range(MG)]
    in_maps = []
    for c in range(N_CORES):
        mg, ng = c // NG, c % NG
        nsl = slice(ng * N_CORE, (ng + 1) * N_CORE)
        in_maps.append({
            "xT": xT_by_mg[mg],
            "wT": np.ascontiguousarray(
                w_eff_T[:, nsl].reshape(KT, P, N_CORE).transpose(1, 0, 2)
                .astype(bf16)),
            "biasb": np.ascontiguousarray(
                np.broadcast_to(bias[nsl], (P, N_CORE))),
        })
    return in_maps


def kernel(x, W, bias, qa, qb, scale_a, scale_b, _trace=False):
    from concourse.bass_utils import run_bass_kernel_spmd

    nc = _get_program()
    in_maps = _make_in_maps(np.asarray(x, dtype=np.float32),
                            np.asarray(W, dtype=np.float32),
                            np.asarray(bias, dtype=np.float32),
                            np.asarray(qa), np.asarray(qb),
                            np.asarray(scale_a), np.asarray(scale_b))
    res = run_bass_kernel_spmd(nc, in_maps, core_ids=list(range(N_CORES)),
                               trace=_trace)
    B, S = 4, 4096
    full = np.empty((MG * M_CORE, NG * N_CORE), dtype=np.float32)
    for c in range(N_CORES):
        mg, ng = c // NG, c % NG
        full[mg * M_CORE:(mg + 1) * M_CORE,
             ng * N_CORE:(ng + 1) * N_CORE] = res.results[c]["out"]
    if _trace:
        kernel._last_results = res
    return full.reshape(B, S, K)


# revision 11
# speedup vs baseline: 6.1861x; 4.3321x over previous
"""
LoRA-Quant-Linear Trainium2 kernel (8 NeuronCores).

Math:  out = x @ W^T + bias + LORA_SCALE * ((x @ a^T) @ b^T)
       a = qa * scale_a  [16, 4096],  b = qb * scale_b  [4096, 16]

Sharding (2 batch-groups x 4 out-column-groups = 8 cores):
  core c = (mg, ng), mg = c // 4, ng = c % 4
    - x rows   [mg*8192 : (mg+1)*8192]  (of B*S = 16384), host-transposed -> xT [4096, 8192]
    - W rows   [ng*1024 : (ng+1)*1024]  (out_features),   host-transposed -> wT [4096, 1024]
  LoRA is folded into the weight chunk on the host (out = x @ (W^T + s*a^T b^T) + bias,
  exact associativity; the fold is 0.4% of the FLOPs).

The GEMM is PE-bound (68.7 GFLOP/core; 1 cycle/row at 2.4 GHz => ~874 us floor),
so the kernel is organized to keep the PE saturated end to end:
  - x and W_eff are cast to bf16 on the host (error ~1.3e-3 of out scale, budget
    is 2e-2).  bf16 halves the HBM stream and enables fast weight load.
  - a short warm-up matmul chain on zeros runs while the first DMAs fill,
    covering the HAM clock-gate ramp (~3.4 us at half clock) and pstate ramp.
  - the first 4 m-slivers are processed kt-major across all 8 PSUM banks, so
    each W k-slice is consumed over ~1.7 us as it lands and the PE never
    outruns the 8 MiB W residency load streaming on the other DMA queue;
    the remaining 60 slivers run mt-major with the PSUM pool
    double-buffering pairs.
  - the last sliver runs as four quarter-width chains, each draining (bias
    add + store) under the next chain's matmuls, shrinking the
    end-of-kernel tail to one quarter drain plus the program drain barrier.
  - x stream on the scalar DMA queue (HWDGE); W residency and out stores
    share the sync queue's hardware DGE without overlapping in time (W done
    ~40 us in, stores start ~60 us in).  gpsimd/SWDGE is avoided: its
    software descriptor build costs ~3.4 us per store and sits on the
    end-of-kernel critical path.
"""

import numpy as np

LORA_SCALE = 32.0 / 16.0

P = 128
K = 4096            # contraction dim (D_in)
KT = K // P         # 32 k-tiles
M_CORE = 8192       # x rows per core
N_CORE = 1024       # out columns per core
MT = M_CORE // P    # 64 m-slivers
NB = 512            # moving free dim per matmul (PSUM bank = 512 f32)
NH = N_CORE // NB   # 2
N_CORES = 8
MG, NG = 2, 4       # core grid
MT_P1 = 4           # slivers processed kt-major in phase 1 (8 PSUM banks)
WARMUP_MM = 24      # warm-up matmuls (~5 us) hidden under the initial fill
P1_OFF = 0          # phase-1 stagger disabled: the 2-HWDGE-queue bus split,
                    # not emission order, binds the early schedule (measured)

_CACHE = {}


def _emit_body(nc, tc, pools, aps, it=""):
    """One full pass of the kernel.  ``it`` uniquifies tile names when the
    body is emitted more than once (timing harness)."""
    from concourse import mybir

    f32 = mybir.dt.float32
    bf16 = mybir.dt.bfloat16
    wpool, cpool, xpool, opool, pspool = pools
    xT, wT, biasb, out_t = aps

    # ---- warm-up: keep the PE busy (and the clock gate open) while the
    # first x slivers + W slices stream in.
    wz = cpool.tile([P, NB], bf16, tag="wz", name=f"wz{it}")
    wz_out = cpool.tile([P, NB], f32, tag="wz_out", name=f"wz_out{it}")
    nc.vector.memset(wz[:], 0.0)
    warm_ps = pspool.tile([P, NB], f32, tag="ps", name=f"ps_warm{it}")
    for i in range(WARMUP_MM):
        nc.tensor.matmul(warm_ps[:], wz[:, :P], wz[:],
                         start=(i == 0), stop=(i == WARMUP_MM - 1))
    nc.vector.tensor_copy(wz_out[:], warm_ps[:])

    # ---- fill: first MT_P1 x slivers on the scalar queue, W residency on
    # sync, bias on the (otherwise idle) gpsimd queue.
    x_sbs = {}
    for mt in range(MT_P1):
        x_sbs[mt] = xpool.tile([P, KT, P], bf16, tag="x", name=f"x_{mt}{it}")
        nc.scalar.dma_start(x_sbs[mt][:], xT[mt])

    # bias is first needed at the phase-1 drain (~60 us in); keep it on the
    # scalar queue behind the phase-1 slivers so it doesn't steal early bus.
    bias_sb = cpool.tile([P, N_CORE], f32, tag="bias", name=f"bias{it}")
    nc.scalar.dma_start(bias_sb[:], biasb)

    w_sb = wpool.tile([P, KT, N_CORE], bf16, tag="w", name=f"w{it}")
    for kt in range(KT):
        nc.sync.dma_start(w_sb[:, kt, :], wT[:, kt, :])

    def drain(mt, pss):
        o_sb = opool.tile([P, N_CORE], f32, tag="o", name=f"o_{mt}{it}")
        for nh in range(NH):
            nc.vector.tensor_add(
                o_sb[:, nh * NB:(nh + 1) * NB],
                pss[nh][:],
                bias_sb[:, nh * NB:(nh + 1) * NB],
            )
        nc.sync.dma_start(out_t[mt], o_sb[:])

    # ---- phase 1: kt-major over the first MT_P1 slivers, diagonally
    # staggered.  Each W k-slice is consumed over ~1.7 us once all chains are
    # active, so the PE never outruns the W residency load, and chain m only
    # needs x sliver m after ~m*P1_OFF*0.85 us.
    ps1 = {(m, h): pspool.tile([P, NB], f32, tag="ps", name=f"ps1_{m}_{h}{it}")
           for m in range(MT_P1) for h in range(NH)}
    for s in range(KT + (MT_P1 - 1) * P1_OFF):
        for m in range(MT_P1):
            kt = s - m * P1_OFF
            if 0 <= kt < KT:
                for h in range(NH):
                    nc.tensor.matmul(
                        ps1[(m, h)][:],
                        x_sbs[m][:, kt, :],
                        w_sb[:, kt, h * NB:(h + 1) * NB],
                        start=(kt == 0), stop=(kt == KT - 1),
                    )
    for m in range(MT_P1):
        drain(m, [ps1[(m, 0)], ps1[(m, 1)]])

    # ---- phase 2: mt-major, streaming x slivers through resident W.
    for mt in range(MT_P1, MT):
        x_sb = xpool.tile([P, KT, P], bf16, tag="x", name=f"x_{mt}{it}")
        nc.scalar.dma_start(x_sb[:], xT[mt])
        pss = [pspool.tile([P, NB], f32, tag="ps", name=f"ps_{mt}_{i}{it}")
               for i in range(NH)]
        if mt < MT - 1:
            for kt in range(KT):
                for nh in range(NH):
                    nc.tensor.matmul(
                        pss[nh][:],
                        x_sb[:, kt, :],
                        w_sb[:, kt, nh * NB:(nh + 1) * NB],
                        start=(kt == 0), stop=(kt == KT - 1),
                    )
            drain(mt, pss)
        else:
            # last sliver: four quarter-width chains, each draining (bias add
            # + store) under the next chain's matmuls, so the end-of-kernel
            # tail is just one quarter drain.
            NQ = NB // 2
            o_sb = opool.tile([P, N_CORE], f32, tag="o", name=f"o_{mt}{it}")
            qss = [pspool.tile([P, NQ], f32, tag="ps", name=f"psq_{q}{it}")
                   for q in range(4)]
            for q in range(4):
                for kt in range(KT):
                    nc.tensor.matmul(
                        qss[q][:],
                        x_sb[:, kt, :],
                        w_sb[:, kt, q * NQ:(q + 1) * NQ],
                        start=(kt == 0), stop=(kt == KT - 1),
                    )
                nc.vector.tensor_add(
                    o_sb[:, q * NQ:(q + 1) * NQ],
                    qss[q][:],
                    bias_sb[:, q * NQ:(q + 1) * NQ],
                )
                nc.sync.dma_start(out_t[mt][:, q * NQ:(q + 1) * NQ],
                                  o_sb[:, q * NQ:(q + 1) * NQ])


def _build_program(reps=1):
    import concourse.tile as tile
    from concourse import bacc, mybir
    from contextlib import ExitStack

    f32 = mybir.dt.float32
    bf16 = mybir.dt.bfloat16

    nc = bacc.Bacc("TRN2", target_bir_lowering=False, debug=False,
                   num_devices=N_CORES)

    # host-pretiled layouts: xT[mt, p, kt, ml] = x[mt*128+ml, kt*128+p]
    # (per-sliver contiguous => 8 KiB/partition DMA lines), and
    # wT[p, kt, n] = W_eff^T[kt*128+p, n].
    xT = nc.dram_tensor("xT", [MT, P, KT, P], bf16, kind="ExternalInput").ap()
    wT = nc.dram_tensor("wT", [P, KT, N_CORE], bf16, kind="ExternalInput").ap()
    biasb = nc.dram_tensor("biasb", [P, N_CORE], f32, kind="ExternalInput").ap()
    out = nc.dram_tensor("out", [M_CORE, N_CORE], f32, kind="ExternalOutput").ap()

    out_t = out.rearrange("(mt p) n -> mt p n", p=P)    # [64, 128, 1024]

    with tile.TileContext(nc) as tc, ExitStack() as ctx:
        pools = (
            ctx.enter_context(tc.tile_pool(name="wres", bufs=1)),
            ctx.enter_context(tc.tile_pool(name="consts", bufs=1)),
            ctx.enter_context(tc.tile_pool(name="xs", bufs=8)),
            ctx.enter_context(tc.tile_pool(name="outs", bufs=4)),
            ctx.enter_context(tc.tile_pool(name="ps", bufs=8, space="PSUM")),
        )
        aps = (xT, wT, biasb, out_t)
        if reps == 1:
            _emit_body(nc, tc, pools, aps)
        else:
            from concourse.mybir import EngineType
            with tc.For_i(0, reps, 1, hint_engines=(EngineType.PE,)):
                _emit_body(nc, tc, pools, aps, it="_r")

    nc.compile()
    return nc


def _get_program():
    if "nc" not in _CACHE:
        _CACHE["nc"] = _build_program()
    return _CACHE["nc"]


def _make_in_maps(x, W, bias, qa, qb, scale_a, scale_b):
    import ml_dtypes

    bf16 = ml_dtypes.bfloat16
    x2 = np.ascontiguousarray(x.reshape(MG * M_CORE, K))
    a_deq = qa.astype(np.float32) * np.float32(scale_a)       # [16, 4096]
    b_deq = qb.astype(np.float32) * np.float32(scale_b)       # [4096, 16]
    # W_eff^T = W^T + s * a^T @ b^T   -> [K, N_full]
    w_eff_T = W.T + np.float32(LORA_SCALE) * (a_deq.T @ b_deq.T)
    bias = bias.astype(np.float32)

    # [mt, ml, kt, p] -> [mt, p, kt, ml], cast to bf16
    xT_by_mg = [np.ascontiguousarray(
                    x2[mg * M_CORE:(mg + 1) * M_CORE, :]
                    .reshape(MT, P, KT, P).transpose(0, 3, 2, 1)
                    .astype(bf16))
                for mg in range(MG)]
    in_maps = []
    for c in range(N_CORES):
        mg, ng = c // NG, c % NG
        nsl = slice(ng * N_CORE, (ng + 1) * N_CORE)
        in_maps.append({
            "xT": xT_by_mg[mg],
            "wT": np.ascontiguousarray(
                w_eff_T[:, nsl].reshape(KT, P, N_CORE).transpose(1, 0, 2)
                .astype(bf16)),
            "biasb": np.ascontiguousarray(
                np.broadcast_to(bias[nsl], (P, N_CORE))),
        })
    return in_maps


def kernel(x, W, bias, qa, qb, scale_a, scale_b, _trace=False):
    from concourse.bass_utils import run_bass_kernel_spmd

    nc = _get_program()
    in_maps = _make_in_maps(np.asarray(x, dtype=np.float32),
                            np.asarray(W, dtype=np.float32),
                            np.asarray(bias, dtype=np.float32),
                            np.asarray(qa), np.asarray(qb),
                            np.asarray(scale_a), np.asarray(scale_b))
    res = run_bass_kernel_spmd(nc, in_maps, core_ids=list(range(N_CORES)),
                               trace=_trace)
    B, S = 4, 4096
    full = np.empty((MG * M_CORE, NG * N_CORE), dtype=np.float32)
    for c in range(N_CORES):
        mg, ng = c // NG, c % NG
        full[mg * M_CORE:(mg + 1) * M_CORE,
             ng * N_CORE:(ng + 1) * N_CORE] = res.results[c]["out"]
    if _trace:
        kernel._last_results = res
    return full.reshape(B, S, K)


# revision 12
# speedup vs baseline: 8.2309x; 1.3305x over previous
"""
LoRA-Quant-Linear Trainium2 kernel (8 NeuronCores).

Math:  out = x @ W^T + bias + LORA_SCALE * ((x @ a^T) @ b^T)
       a = qa * scale_a  [16, 4096],  b = qb * scale_b  [4096, 16]

Sharding (2 batch-groups x 4 out-column-groups = 8 cores):
  core c = (mg, ng), mg = c // 4, ng = c % 4
    - x rows   [mg*8192 : (mg+1)*8192]  (of B*S = 16384), host-transposed -> xT [4096, 8192]
    - W rows   [ng*1024 : (ng+1)*1024]  (out_features),   host-transposed -> wT [4096, 1024]
  LoRA is folded into the weight chunk on the host (out = x @ (W^T + s*a^T b^T) + bias,
  exact associativity; the fold is 0.4% of the FLOPs).

The GEMM is PE-bound (68.7 GFLOP/core; 1 cycle/row at 2.4 GHz => ~874 us floor),
so the kernel is organized to keep the PE saturated end to end:
  - x and W_eff are cast to bf16 on the host (error ~1.3e-3 of out scale, budget
    is 2e-2).  bf16 halves the HBM stream and enables fast weight load.
  - a short warm-up matmul chain on zeros runs while the first DMAs fill,
    covering the HAM clock-gate ramp (~3.4 us at half clock) and pstate ramp.
  - the first 4 m-slivers are processed kt-major across all 8 PSUM banks, so
    each W k-slice is consumed over ~1.7 us as it lands and the PE never
    outruns the 8 MiB W residency load streaming on the other DMA queue;
    the remaining 60 slivers run mt-major with the PSUM pool
    double-buffering pairs.
  - the last sliver runs as four quarter-width chains, each draining (bias
    add + store) under the next chain's matmuls, shrinking the
    end-of-kernel tail to one quarter drain plus the program drain barrier.
  - x stream on the scalar DMA queue (HWDGE); W residency and out stores
    share the sync queue's hardware DGE without overlapping in time (W done
    ~40 us in, stores start ~60 us in).  gpsimd/SWDGE is avoided: its
    software descriptor build costs ~3.4 us per store and sits on the
    end-of-kernel critical path.
"""

import numpy as np

LORA_SCALE = 32.0 / 16.0

P = 128
K = 4096            # contraction dim (D_in)
KT = K // P         # 32 k-tiles
M_CORE = 8192       # x rows per core
N_CORE = 1024       # out columns per core
MT = M_CORE // P    # 64 m-slivers
NB = 512            # moving free dim per matmul (PSUM bank = 512 f32)
NH = N_CORE // NB   # 2
N_CORES = 8
MG, NG = 2, 4       # core grid
MT_P1 = 4           # slivers processed kt-major in phase 1 (8 PSUM banks)
WARMUP_MM = 20      # warm-up matmuls (~4.3 us) hidden under the initial fill
P1_OFF = 0          # phase-1 stagger disabled: the 2-HWDGE-queue bus split,
                    # not emission order, binds the early schedule (measured)

_CACHE = {}


def _emit_body(nc, tc, pools, aps, it=""):
    """One full pass of the kernel.  ``it`` uniquifies tile names when the
    body is emitted more than once (timing harness)."""
    from concourse import mybir

    f32 = mybir.dt.float32
    bf16 = mybir.dt.bfloat16
    wpool, cpool, xpool, opool, pspool = pools
    xT, wT, biasb, out_t = aps

    # ---- warm-up: keep the PE busy (and the clock gate open) while the
    # first x slivers + W slices stream in.
    wz = cpool.tile([P, NB], bf16, tag="wz", name=f"wz{it}")
    wz_out = cpool.tile([P, NB], f32, tag="wz_out", name=f"wz_out{it}")
    nc.vector.memset(wz[:], 0.0)
    warm_ps = pspool.tile([P, NB], f32, tag="ps", name=f"ps_warm{it}")
    for i in range(WARMUP_MM):
        nc.tensor.matmul(warm_ps[:], wz[:, :P], wz[:],
                         start=(i == 0), stop=(i == WARMUP_MM - 1))
    nc.vector.tensor_copy(wz_out[:], warm_ps[:])

    # ---- fill: first MT_P1 x slivers on the scalar queue, W residency on
    # sync, bias on the (otherwise idle) gpsimd queue.
    # phase-1 slivers arrive in kt-halves: chain m's early k-steps only need
    # the first half, so it can start ~half a sliver-DMA sooner.
    KH = KT // 2
    x_sbs = {}
    for mt in range(MT_P1):
        x_sbs[mt] = xpool.tile([P, KT, P], bf16, tag="x", name=f"x_{mt}{it}")
        nc.scalar.dma_start(x_sbs[mt][:, :KH, :], xT[mt, :, :KH, :])
        nc.scalar.dma_start(x_sbs[mt][:, KH:, :], xT[mt, :, KH:, :])

    # bias is first needed at the phase-1 drain (~60 us in); keep it on the
    # scalar queue behind the phase-1 slivers so it doesn't steal early bus.
    bias_sb = cpool.tile([P, N_CORE], f32, tag="bias", name=f"bias{it}")
    nc.scalar.dma_start(bias_sb[:], biasb)

    w_sb = wpool.tile([P, KT, N_CORE], bf16, tag="w", name=f"w{it}")
    for kt in range(KT):
        nc.sync.dma_start(w_sb[:, kt, :], wT[:, kt, :])

    def drain(mt, pss):
        o_sb = opool.tile([P, N_CORE], f32, tag="o", name=f"o_{mt}{it}")
        for nh in range(NH):
            nc.vector.tensor_add(
                o_sb[:, nh * NB:(nh + 1) * NB],
                pss[nh][:],
                bias_sb[:, nh * NB:(nh + 1) * NB],
            )
        nc.sync.dma_start(out_t[mt], o_sb[:])

    # ---- phase 1: kt-major over the first MT_P1 slivers, diagonally
    # staggered.  Each W k-slice is consumed over ~1.7 us once all chains are
    # active, so the PE never outruns the W residency load, and chain m only
    # needs x sliver m after ~m*P1_OFF*0.85 us.
    ps1 = {(m, h): pspool.tile([P, NB], f32, tag="ps", name=f"ps1_{m}_{h}{it}")
           for m in range(MT_P1) for h in range(NH)}
    for s in range(KT + (MT_P1 - 1) * P1_OFF):
        for m in range(MT_P1):
            kt = s - m * P1_OFF
            if 0 <= kt < KT:
                for h in range(NH):
                    nc.tensor.matmul(
                        ps1[(m, h)][:],
                        x_sbs[m][:, kt, :],
                        w_sb[:, kt, h * NB:(h + 1) * NB],
                        start=(kt == 0), stop=(kt == KT - 1),
                    )
    for m in range(MT_P1):
        drain(m, [ps1[(m, 0)], ps1[(m, 1)]])

    # ---- phase 2: mt-major, streaming x slivers through resident W.
    for mt in range(MT_P1, MT):
        x_sb = xpool.tile([P, KT, P], bf16, tag="x", name=f"x_{mt}{it}")
        nc.scalar.dma_start(x_sb[:], xT[mt])
        pss = [pspool.tile([P, NB], f32, tag="ps", name=f"ps_{mt}_{i}{it}")
               for i in range(NH)]
        if mt < MT - 1:
            for kt in range(KT):
                for nh in range(NH):
                    nc.tensor.matmul(
                        pss[nh][:],
                        x_sb[:, kt, :],
                        w_sb[:, kt, nh * NB:(nh + 1) * NB],
                        start=(kt == 0), stop=(kt == KT - 1),
                    )
            drain(mt, pss)
        else:
            # last sliver: four quarter-width chains, each draining (bias add
            # + store) under the next chain's matmuls, so the end-of-kernel
            # tail is just one quarter drain.
            NQ = NB // 2
            o_sb = opool.tile([P, N_CORE], f32, tag="o", name=f"o_{mt}{it}")
            qss = [pspool.tile([P, NQ], f32, tag="ps", name=f"psq_{q}{it}")
                   for q in range(4)]
            for q in range(4):
                for kt in range(KT):
                    nc.tensor.matmul(
                        qss[q][:],
                        x_sb[:, kt, :],
                        w_sb[:, kt, q * NQ:(q + 1) * NQ],
                        start=(kt == 0), stop=(kt == KT - 1),
                    )
                nc.vector.tensor_add(
                    o_sb[:, q * NQ:(q + 1) * NQ],
                    qss[q][:],
                    bias_sb[:, q * NQ:(q + 1) * NQ],
                )
                nc.sync.dma_start(out_t[mt][:, q * NQ:(q + 1) * NQ],
                                  o_sb[:, q * NQ:(q + 1) * NQ])


def _build_program(reps=1):
    import concourse.tile as tile
    from concourse import bacc, mybir
    from contextlib import ExitStack

    f32 = mybir.dt.float32
    bf16 = mybir.dt.bfloat16

    nc = bacc.Bacc("TRN2", target_bir_lowering=False, debug=False,
                   num_devices=N_CORES)

    # host-pretiled layouts: xT[mt, p, kt, ml] = x[mt*128+ml, kt*128+p]
    # (per-sliver contiguous => 8 KiB/partition DMA lines), and
    # wT[p, kt, n] = W_eff^T[kt*128+p, n].
    xT = nc.dram_tensor("xT", [MT, P, KT, P], bf16, kind="ExternalInput").ap()
    wT = nc.dram_tensor("wT", [P, KT, N_CORE], bf16, kind="ExternalInput").ap()
    biasb = nc.dram_tensor("biasb", [P, N_CORE], f32, kind="ExternalInput").ap()
    out = nc.dram_tensor("out", [M_CORE, N_CORE], f32, kind="ExternalOutput").ap()

    out_t = out.rearrange("(mt p) n -> mt p n", p=P)    # [64, 128, 1024]

    with tile.TileContext(nc) as tc, ExitStack() as ctx:
        pools = (
            ctx.enter_context(tc.tile_pool(name="wres", bufs=1)),
            ctx.enter_context(tc.tile_pool(name="consts", bufs=1)),
            ctx.enter_context(tc.tile_pool(name="xs", bufs=8)),
            ctx.enter_context(tc.tile_pool(name="outs", bufs=4)),
            ctx.enter_context(tc.tile_pool(name="ps", bufs=8, space="PSUM")),
        )
        aps = (xT, wT, biasb, out_t)
        if reps == 1:
            _emit_body(nc, tc, pools, aps)
        else:
            from concourse.mybir import EngineType
            with tc.For_i(0, reps, 1, hint_engines=(EngineType.PE,)):
                _emit_body(nc, tc, pools, aps, it="_r")

    nc.compile()
    return nc


def _get_program():
    if "nc" not in _CACHE:
        _CACHE["nc"] = _build_program()
    return _CACHE["nc"]


def _make_in_maps(x, W, bias, qa, qb, scale_a, scale_b):
    import ml_dtypes

    bf16 = ml_dtypes.bfloat16
    x2 = np.ascontiguousarray(x.reshape(MG * M_CORE, K))
    a_deq = qa.astype(np.float32) * np.float32(scale_a)       # [16, 4096]
    b_deq = qb.astype(np.float32) * np.float32(scale_b)       # [4096, 16]
    # W_eff^T = W^T + s * a^T @ b^T   -> [K, N_full]
    w_eff_T = W.T + np.float32(LORA_SCALE) * (a_deq.T @ b_deq.T)
    bias = bias.astype(np.float32)

    # [mt, ml, kt, p] -> [mt, p, kt, ml], cast to bf16
    xT_by_mg = [np.ascontiguousarray(
                    x2[mg * M_CORE:(mg + 1) * M_CORE, :]
                    .reshape(MT, P, KT, P).transpose(0, 3, 2, 1)
                    .astype(bf16))
                for mg in range(MG)]
    in_maps = []
    for c in range(N_CORES):
        mg, ng = c // NG, c % NG
        nsl = slice(ng * N_CORE, (ng + 1) * N_CORE)
        in_maps.append({
            "xT": xT_by_mg[mg],
            "wT": np.ascontiguousarray(
                w_eff_T[:, nsl].reshape(KT, P, N_CORE).transpose(1, 0, 2)
                .astype(bf16)),
            "biasb": np.ascontiguousarray(
                np.broadcast_to(bias[nsl], (P, N_CORE))),
        })
    return in_maps


def kernel(x, W, bias, qa, qb, scale_a, scale_b, _trace=False):
    from concourse.bass_utils import run_bass_kernel_spmd

    nc = _get_program()
    in_maps = _make_in_maps(np.asarray(x, dtype=np.float32),
                            np.asarray(W, dtype=np.float32),
                            np.asarray(bias, dtype=np.float32),
                            np.asarray(qa), np.asarray(qb),
                            np.asarray(scale_a), np.asarray(scale_b))
    res = run_bass_kernel_spmd(nc, in_maps, core_ids=list(range(N_CORES)),
                               trace=_trace)
    B, S = 4, 4096
    full = np.empty((MG * M_CORE, NG * N_CORE), dtype=np.float32)
    for c in range(N_CORES):
        mg, ng = c // NG, c % NG
        full[mg * M_CORE:(mg + 1) * M_CORE,
             ng * N_CORE:(ng + 1) * N_CORE] = res.results[c]["out"]
    if _trace:
        kernel._last_results = res
    return full.reshape(B, S, K)
